# revision 23
# baseline (speedup 1.0000x reference)
"""Trainium2 Bass kernel for KeyframeSelectionNetwork.

Math (per (b, v) video of T=64 frames, F=1024 features):
  GCN with self-loops + one edge (frame0 -> frame1), symmetric norm:
    out[t] = x[t] @ W_gcn                      for t != 1
    out[1] = (0.5*x[1] + (1/sqrt(2))*x[0]) @ W_gcn
  pooled = max_t out[t] + b_gcn
  h = relu(pooled.reshape(B, V*F) @ W1 + b1)  -> [B, 256]
  key = sigmoid(h @ W2 + b2)                  -> [B, V, T]

Strategy: data-parallel over batch across 8 cores (8 videos' batches each).
Host-side sharding prep: X is cast to bf16 and transposed to X^T [F, NLOC]
with v-major node order (node = (v*BL + b)*T + t), weights cast to bf16,
and b_gcn is folded into b1 (b1' = b1 + tile(b_gcn, V) @ W1 — valid since
max_t commutes with the constant shift).  Per core:
  - per 512-node chunk: one 1MB HWDGE DMA of X^T [128, KC=8, 512] bf16 on
    the SP ring; W1/W2/b* ride the ACT ring so X never queues behind them.
  - GCN edge combine = two strided DVE ops on the t=0/t=1 columns
    (x1 <- 0.5*x1 + (1/sqrt2)*x0), linear so it commutes with the matmul.
  - chunk 0 fast-start: per-k 128KB X slices interleaved with per-k W_gcn
    loads; k-outer matmuls in two 4-PSUM-bank half passes, so the PE
    starts ~2us in instead of waiting for the full chunk + all weights.
  - chunks 1-7: PE matmul Y.T[j, nodes] = W_gcn[kblk, jblk].T @ X.T,
    N=512 moving, PSUM-accumulated over k (j-outer, 4 banks cycling).
  - max-pool over t via one 3D-AP reduce_max per PSUM tile -> pooledT.
  - v-major order means chunk c completes pooled video v=c, so the MLP's
    first-layer matmuls for v=c-1 interleave into the chunk stream; only
    video 7's slice + the tiny second layer remain in the tail.
"""

import sys

sys.path.insert(0, "/opt/trn_rl_repo")

import numpy as np
import ml_dtypes

BF16 = ml_dtypes.bfloat16
FP8 = ml_dtypes.float8_e4m3  # matches mybir.dt.float8e4 (TRN FP8_EXP4)

B, V, T, F = 64, 8, 64, 1024
NCORES = 8
BL = B // NCORES  # batches per core
NLOC = BL * V * T  # nodes per core (4096)
H1 = 256
OUT = V * T  # 512
P = 128
CHUNK = 512  # nodes per chunk
NCH = NLOC // CHUNK  # 8 (one chunk = one v across all local batches)
KC = F // P  # 8 contraction chunks
JC = F // P  # 8 output-feature chunks
GR = CHUNK // T  # pool groups per chunk = 8

CFG = dict(
    x_bufs=3,
    psum_bufs=5,
    double_row=True,
    mlp_interleave=False,
)

_STATE = None


def _build_nc(cfg, reps=1):
    import concourse.bacc as bacc
    import concourse.tile as tile
    from concourse import mybir

    f32 = mybir.dt.float32
    bf16 = mybir.dt.bfloat16
    fp8 = mybir.dt.float8e4
    AF = mybir.ActivationFunctionType
    ALU = mybir.AluOpType
    DR = mybir.MatmulPerfMode.DoubleRow

    nc = bacc.Bacc(None, target_bir_lowering=False, debug=False)
    # X^T, transposed + fp8-cast host-side during sharding (v-major nodes)
    x_d = nc.dram_tensor("videosT", [F, NLOC], fp8, kind="ExternalInput")
    wg_d = nc.dram_tensor("W_gcn", [F, F], fp8, kind="ExternalInput")
    w1_d = nc.dram_tensor("W1", [V * F, H1], bf16, kind="ExternalInput")
    b1_d = nc.dram_tensor("b1", [H1], f32, kind="ExternalInput")  # b_gcn folded
    w2_d = nc.dram_tensor("W2", [H1, OUT], bf16, kind="ExternalInput")
    b2_d = nc.dram_tensor("b2", [OUT], f32, kind="ExternalInput")
    id8_d = nc.dram_tensor("id8", [BL, BL], f32, kind="ExternalInput")
    if reps == 1:
        out_d = nc.dram_tensor("out", [BL, OUT], f32, kind="ExternalOutput")
    else:
        # distinct per-rep outputs so DCE can't drop repeated workloads
        out_d = nc.dram_tensor("out", [reps, BL, OUT], f32, kind="ExternalOutput")

    SQ2 = 1.4142135623730951

    with tile.TileContext(nc) as tc:
      with (
          tc.tile_pool(name="const", bufs=1) as const,
          tc.tile_pool(name="xpool", bufs=cfg.get("x_bufs", 3)) as xpool,
      ):
        for _rep in range(reps):
            # ---- resident tiles ----
            wg_sb = const.tile([P, KC, F], fp8)
            w1_sb = const.tile([P, V * KC, H1], bf16)
            w2_sb = const.tile([P, 2, OUT], bf16)
            b1_sb = const.tile([1, H1], f32)
            b2_sb = const.tile([1, OUT], f32)
            ones_sb = const.tile([1, BL], f32)
            id8_sb = const.tile([BL, BL], f32)
            pooled_m = const.tile([P, JC, BL * V], bf16)
            xt0 = const.tile([P, KC, CHUNK], fp8, tag="x0")

            xt_tiles = {}

            def fix_edges(ap_kslab):
                # x1 <- (sqrt2 * x0 + x1) * 0.5 on the t=0/1 columns
                xv = ap_kslab.rearrange("p k (g t) -> p k g t", t=T)
                x0 = xv[:, :, :, 0:1]
                x1 = xv[:, :, :, 1:2]
                nc.vector.scalar_tensor_tensor(x1, x0, SQ2, x1, ALU.mult, ALU.add)
                nc.vector.tensor_scalar_mul(x1, x1, 0.5)

            def dma_x(c):
                t = xpool.tile([P, KC, CHUNK], fp8, tag="x")
                src = x_d[:, c * CHUNK : (c + 1) * CHUNK].rearrange(
                    "(k p) n -> p k n", p=P
                )
                nc.sync.dma_start(t[:], src)
                xt_tiles[c] = t

            # ---- prologue: chunk 0 on the SP ring (first k-pair slab
            # separately so the first DR matmul can start early), W_gcn
            # k-pair slabs on the ACT ring (no head-of-line blocking) ----
            nc.sync.dma_start(
                xt0[:, 0:2, :],
                x_d[0 : 2 * P, 0:CHUNK].rearrange("(k p) n -> p k n", p=P),
            )
            nc.sync.dma_start(
                xt0[:, 2:KC, :],
                x_d[2 * P : F, 0:CHUNK].rearrange("(k p) n -> p k n", p=P),
            )
            for q in range(KC // 2):
                nc.scalar.dma_start(
                    wg_sb[:, 2 * q : 2 * q + 2, :],
                    wg_d[2 * q * P : (2 * q + 2) * P, :].rearrange(
                        "(k p) f -> p k f", p=P
                    ),
                )
            fix_edges(xt0[:, 0:2, :])
            fix_edges(xt0[:, 2:KC, :])
            dma_x(1)
            fix_edges(xt_tiles[1][:])
            dma_x(2)
            fix_edges(xt_tiles[2][:])
            nc.scalar.dma_start(b1_sb[:], b1_d.rearrange("(o n) -> o n", o=1))
            nc.scalar.dma_start(b2_sb[:], b2_d.rearrange("(o n) -> o n", o=1))
            nc.scalar.dma_start(id8_sb[:], id8_d[:])
            nc.vector.memset(ones_sb[:], 1.0)
            nc.scalar.dma_start(
                w1_sb[:], w1_d[:].rearrange("(i p) n -> p i n", p=P)
            )
            nc.scalar.dma_start(
                w2_sb[:], w2_d[:].rearrange("(m p) n -> p m n", p=P)
            )

            with (
                tc.tile_pool(
                    name=f"mpsum{_rep}", bufs=cfg.get("psum_bufs", 4), space="PSUM"
                ) as mpsum,
                tc.tile_pool(name=f"lpsum{_rep}", bufs=1, space="PSUM") as lpsum,
            ):
                hp = lpsum.tile([BL, H1], f32, tag="hp")

                def mlp_slice(c):
                    # first-layer matmuls for video v=c (pooled groups
                    # c*GR..(c+1)*GR are batches 0..BL-1 of video c)
                    for fc in range(KC):
                        i = c * KC + fc
                        nc.tensor.matmul(
                            hp[:],
                            pooled_m[:, fc, c * GR : (c + 1) * GR],
                            w1_sb[:, i, :],
                            start=(i == 0),
                            stop=False,
                        )

                # ---- chunks: j-outer Y-phase (fp8 DoubleRow, K=256 per
                # matmul) + reduce_max straight to bf16 + interleaved MLP ----
                xt_tiles[0] = xt0
                NQ = KC // 2
                for c in range(NCH):
                    if c + 2 < NCH and c > 0:
                        dma_x(c + 2)
                        fix_edges(xt_tiles[c + 2][:])
                    xt = xt_tiles.pop(c)
                    for j in range(JC):
                        yp = mpsum.tile([P, CHUNK], f32, tag="yp")
                        if cfg.get("double_row", True):
                            for q in range(NQ):
                                nc.tensor.matmul(
                                    yp[:],
                                    wg_sb[
                                        :, 2 * q : 2 * q + 2, j * P : (j + 1) * P
                                    ],
                                    xt[:, 2 * q : 2 * q + 2, :],
                                    start=(q == 0),
                                    stop=(q == NQ - 1),
                                    perf_mode=DR,
                                )
                        else:
                            for k in range(KC):
                                nc.tensor.matmul(
                                    yp[:],
                                    wg_sb[:, k, j * P : (j + 1) * P],
                                    xt[:, k, :],
                                    start=(k == 0),
                                    stop=(k == KC - 1),
                                )
                        nc.vector.reduce_max(
                            pooled_m[:, j, c * GR : (c + 1) * GR],
                            yp[:].rearrange("p (g t) -> p g t", t=T),
                            axis=mybir.AxisListType.X,
                        )
                    if c > 0 and cfg.get("mlp_interleave", True):
                        mlp_slice(c - 1)

                # ---- tail: last MLP slice, bias, relu, layer 2 ----
                if not cfg.get("mlp_interleave", True):
                    for c in range(NCH - 1):
                        mlp_slice(c)
                mlp_slice(NCH - 1)
                nc.tensor.matmul(hp[:], ones_sb[:], b1_sb[:], start=False, stop=True)
                h_sb = const.tile([BL, H1], f32)
                nc.scalar.activation(h_sb[:], hp[:], AF.Relu)

                ht_sb = const.tile([P, 2, BL], bf16)
                for m in range(2):
                    thp = lpsum.tile([P, BL], f32, tag="thp")
                    nc.tensor.transpose(
                        thp[:], h_sb[:, m * P : (m + 1) * P], id8_sb[:]
                    )
                    nc.vector.tensor_copy(ht_sb[:, m, :], thp[:])

                op = lpsum.tile([BL, OUT], f32, tag="op")
                for m in range(2):
                    nc.tensor.matmul(
                        op[:], ht_sb[:, m, :], w2_sb[:, m, :], start=(m == 0),
                        stop=False,
                    )
                nc.tensor.matmul(op[:], ones_sb[:], b2_sb[:], start=False, stop=True)
                o_sb = const.tile([BL, OUT], f32)
                nc.scalar.activation(o_sb[:], op[:], AF.Sigmoid)
                nc.sync.dma_start(
                    out_d[:] if reps == 1 else out_d[_rep], o_sb[:]
                )

    nc.compile()
    return nc


def _get_state(cfg=None):
    global _STATE
    if _STATE is None:
        _STATE = _build_nc(cfg or CFG)
    return _STATE


def make_in_maps(videos, W_gcn, b_gcn, W1, b1, W2, b2):
    videos = np.asarray(videos, dtype=np.float32)
    W1f = np.asarray(W1, dtype=np.float32)
    b_gcn = np.asarray(b_gcn, dtype=np.float32)
    # fold b_gcn into b1: relu((pooled + bg)@W1 + b1) = relu(pooled@W1 + b1')
    b1p = (np.asarray(b1, dtype=np.float32) + np.tile(b_gcn, V) @ W1f).astype(
        np.float32
    )
    id8 = np.eye(BL, dtype=np.float32)
    common = {
        "W_gcn": np.asarray(W_gcn, dtype=np.float32).astype(FP8),
        "W1": W1f.astype(BF16),
        "b1": b1p,
        "W2": np.asarray(W2, dtype=np.float32).astype(BF16),
        "b2": np.asarray(b2, dtype=np.float32),
        "id8": id8,
    }
    in_maps = []
    for i in range(NCORES):
        m = dict(common)
        # shard over batch; v-major node order; cast fp8; transpose to [F, NLOC]
        xc = videos[i * BL : (i + 1) * BL]  # [BL, V, T, F]
        xv = xc.transpose(1, 0, 2, 3).reshape(NLOC, F).astype(FP8)
        m["videosT"] = np.ascontiguousarray(xv.T)
        in_maps.append(m)
    return in_maps


_RUNNER = None


def _make_runner(nc):
    """Cached multi-core PJRT runner (mirrors bass2jax.run_bass_via_pjrt but
    jits once so repeated calls don't re-trace)."""
    import jax
    import numpy as _np
    from jax.experimental.shard_map import shard_map
    from jax.sharding import Mesh, PartitionSpec
    from concourse import bass2jax, mybir

    bass2jax.install_neuronx_cc_hook()
    assert nc.dbg_addr is None
    partition_name = (
        nc.partition_id_tensor.name if nc.partition_id_tensor is not None else None
    )

    in_names, out_names, out_avals, zero_outs = [], [], [], []
    for alloc in nc.m.functions[0].allocations:
        if not isinstance(alloc, mybir.MemoryLocationSet):
            continue
        name = alloc.memorylocations[0].name
        if alloc.kind == "ExternalInput":
            if name != partition_name:
                in_names.append(name)
        elif alloc.kind == "ExternalOutput":
            out_names.append(name)
            shape = tuple(alloc.tensor_shape)
            dtype = mybir.dt.np(alloc.dtype)
            out_avals.append(jax.core.ShapedArray(shape, dtype))
            zero_outs.append(_np.zeros(shape, dtype))
    n_params = len(in_names)
    n_outs = len(out_avals)
    all_names = in_names + out_names
    if partition_name is not None:
        all_names = all_names + [partition_name]

    def _body(*args):
        operands = list(args)
        if partition_name is not None:
            operands.append(bass2jax.partition_id_tensor())
        outs = bass2jax._bass_exec_p.bind(
            *operands,
            out_avals=tuple(out_avals),
            in_names=tuple(all_names),
            out_names=tuple(out_names),
            lowering_input_output_aliases=(),
            sim_require_finite=True,
            sim_require_nnan=True,
            nc=nc,
        )
        return tuple(outs)

    devices = jax.devices()[:NCORES]
    mesh = Mesh(np.asarray(devices), ("core",))
    in_specs = (PartitionSpec("core"),) * (n_params + n_outs)
    out_specs = (PartitionSpec("core"),) * n_outs
    sharded = jax.jit(
        shard_map(
            _body, mesh=mesh, in_specs=in_specs, out_specs=out_specs, check_rep=False
        ),
        keep_unused=True,
    )

    def run(in_maps, device_inputs=None):
        if device_inputs is None:
            device_inputs = prep(in_maps)
        out_arrs = sharded(*device_inputs)
        jax.block_until_ready(out_arrs)
        return [
            {
                name: _np.asarray(out_arrs[i]).reshape(NCORES, *out_avals[i].shape)[c]
                for i, name in enumerate(out_names)
            }
            for c in range(NCORES)
        ]

    def prep(in_maps):
        from jax.sharding import NamedSharding

        concat_in = [
            _np.concatenate([_np.asarray(in_maps[c][nm]) for c in range(NCORES)], 0)
            for nm in in_names
        ]
        concat_zeros = [
            _np.zeros((NCORES * z.shape[0], *z.shape[1:]), z.dtype) for z in zero_outs
        ]
        sh = NamedSharding(mesh, PartitionSpec("core"))
        arrs = [jax.device_put(a, sh) for a in concat_in + concat_zeros]
        jax.block_until_ready(arrs)
        return arrs

    return run, prep


def _get_runner():
    global _RUNNER
    if _RUNNER is None:
        _RUNNER = _make_runner(_get_state())
    return _RUNNER


def run_spmd(in_maps, device_inputs=None):
    run, _ = _get_runner()
    return run(in_maps, device_inputs)


def prep_inputs(in_maps):
    _, prep = _get_runner()
    return prep(in_maps)


def kernel(videos, W_gcn, b_gcn, W1, b1, W2, b2):
    in_maps = make_in_maps(videos, W_gcn, b_gcn, W1, b1, W2, b2)
    results = run_spmd(in_maps)
    out = np.stack([results[i]["out"] for i in range(NCORES)])  # [8, 8, 512]
    return out.reshape(B, OUT).reshape(B, V, T).astype(np.float32)


# revision 35
# speedup vs baseline: 1.0101x; 1.0101x over previous
"""Trainium2 Bass kernel for KeyframeSelectionNetwork.

Math (per (b, v) video of T=64 frames, F=1024 features):
  GCN with self-loops + one edge (frame0 -> frame1), symmetric norm:
    out[t] = x[t] @ W_gcn                      for t != 1
    out[1] = (0.5*x[1] + (1/sqrt(2))*x[0]) @ W_gcn
  pooled = max_t out[t] + b_gcn
  h = relu(pooled.reshape(B, V*F) @ W1 + b1)  -> [B, 256]
  key = sigmoid(h @ W2 + b2)                  -> [B, V, T]

Strategy: data-parallel over batch across 8 cores (8 videos' batches each).
Host-side sharding prep: X is cast to fp8 e4m3 and transposed to X^T
[F, NLOC] with v-major node order (node = (v*BL + b)*T + t), W_gcn cast to
fp8, W1/W2 to bf16, and b_gcn folded into b1 (b1' = b1 + tile(b_gcn,V)@W1 —
valid since max_t commutes with the constant shift).  Per core:
  - per 512-node chunk: one 256KB HWDGE DMA of X^T [128, KC=8, 512] fp8 on
    the SP ring; W_gcn/W1/W2/b* ride the ACT ring so X never queues behind
    them.
  - GCN edge combine = two strided DVE ops on the t=0/t=1 columns
    (x1 <- 0.5*x1 + (1/sqrt2)*x0), linear so it commutes with the matmul.
  - Y-phase: fp8 DoubleRow PE matmuls (2 fp8 weights per cell, K=256 per
    matmul, N=512 moving => 2x bf16 FLOP rate), PSUM-accumulated over 4
    k-pair blocks, j-outer with 4 PSUM banks cycling.  End-to-end rel err
    vs the fp32 reference is ~8.9e-3 (gate 2e-2).
  - max-pool over t via one 3D-AP reduce_max per PSUM tile, writing bf16
    pooled_m directly (no separate cast op).
  - the MLP (bf16) runs after the full DR stream: interleaving bf16
    matmuls between DR accumulation groups at chunk boundaries hangs the
    PE on TRN2 (NRT_EXEC_UNIT_UNRECOVERABLE) — do not re-interleave.
"""

import sys

sys.path.insert(0, "/opt/trn_rl_repo")

import numpy as np
import ml_dtypes

BF16 = ml_dtypes.bfloat16
FP8 = ml_dtypes.float8_e4m3  # matches mybir.dt.float8e4 (TRN FP8_EXP4)

B, V, T, F = 64, 8, 64, 1024
NCORES = 8
BL = B // NCORES  # batches per core
NLOC = BL * V * T  # nodes per core (4096)
H1 = 256
OUT = V * T  # 512
P = 128
CHUNK = 512  # nodes per chunk
NCH = NLOC // CHUNK  # 8 (one chunk = one v across all local batches)
KC = F // P  # 8 contraction chunks
JC = F // P  # 8 output-feature chunks
GR = CHUNK // T  # pool groups per chunk = 8

CFG = dict(
    x_bufs=3,
    psum_bufs=4,
    double_row=True,
    mlp_interleave=False,
)

_STATE = None


def _build_nc(cfg, reps=1):
    import concourse.bacc as bacc
    import concourse.tile as tile
    from concourse import mybir

    f32 = mybir.dt.float32
    bf16 = mybir.dt.bfloat16
    fp8 = mybir.dt.float8e4
    AF = mybir.ActivationFunctionType
    ALU = mybir.AluOpType
    DR = mybir.MatmulPerfMode.DoubleRow

    nc = bacc.Bacc(None, target_bir_lowering=False, debug=False)
    # X^T, transposed + fp8-cast host-side during sharding (v-major nodes)
    x_d = nc.dram_tensor("videosT", [F, NLOC], fp8, kind="ExternalInput")
    wg_d = nc.dram_tensor("W_gcn", [F, F], fp8, kind="ExternalInput")
    w1_d = nc.dram_tensor("W1", [V * F, H1], bf16, kind="ExternalInput")
    b1_d = nc.dram_tensor("b1", [H1], f32, kind="ExternalInput")  # b_gcn folded
    w2_d = nc.dram_tensor("W2", [H1, OUT], bf16, kind="ExternalInput")
    b2_d = nc.dram_tensor("b2", [OUT], f32, kind="ExternalInput")
    id8_d = nc.dram_tensor("id8", [BL, BL], f32, kind="ExternalInput")
    if reps == 1:
        out_d = nc.dram_tensor("out", [BL, OUT], f32, kind="ExternalOutput")
    else:
        # distinct per-rep outputs so DCE can't drop repeated workloads
        out_d = nc.dram_tensor("out", [reps, BL, OUT], f32, kind="ExternalOutput")

    SQ2 = 1.4142135623730951

    with tile.TileContext(nc) as tc:
      with (
          tc.tile_pool(name="const", bufs=1) as const,
          tc.tile_pool(name="xpool", bufs=cfg.get("x_bufs", 3)) as xpool,
      ):
        for _rep in range(reps):
            # ---- resident tiles ----
            wg_sb = const.tile([P, KC, F], fp8)
            w1_sb = const.tile([P, V * KC, H1], bf16)
            w2_sb = const.tile([P, 2, OUT], bf16)
            b1_sb = const.tile([1, H1], f32)
            b2_sb = const.tile([1, OUT], f32)
            ones_sb = const.tile([1, BL], f32)
            id8_sb = const.tile([BL, BL], f32)
            pooled_m = const.tile([P, JC, BL * V], bf16)
            xt0 = const.tile([P, KC, CHUNK], fp8, tag="x0")

            xt_tiles = {}

            def fix_edges(ap_kslab):
                # x1 <- (sqrt2 * x0 + x1) * 0.5 on the t=0/1 columns
                # (scalar_tensor_tensor is DVE-only on TRN2; the Pool engine
                # rejects the opcode at codegen)
                xv = ap_kslab.rearrange("p k (g t) -> p k g t", t=T)
                x0 = xv[:, :, :, 0:1]
                x1 = xv[:, :, :, 1:2]
                nc.vector.scalar_tensor_tensor(x1, x0, SQ2, x1, ALU.mult, ALU.add)
                nc.vector.tensor_scalar_mul(x1, x1, 0.5)

            def dma_x(c):
                t = xpool.tile([P, KC, CHUNK], fp8, tag="x")
                src = x_d[:, c * CHUNK : (c + 1) * CHUNK].rearrange(
                    "(k p) n -> p k n", p=P
                )
                nc.sync.dma_start(t[:], src)
                xt_tiles[c] = t

            # ---- prologue: chunk 0 on the SP ring, W_gcn k-pair slabs on
            # the ACT ring (no head-of-line blocking).  NOTE: splitting xt0
            # or the q=0 wg slab into partial-tile DMAs was tried for a
            # faster start and produced an intermittent NaN (suspected
            # read-before-write race on the partial slices) — keep these as
            # whole-region DMAs. ----
            nc.sync.dma_start(
                xt0[:], x_d[:, 0:CHUNK].rearrange("(k p) n -> p k n", p=P)
            )
            for q in range(KC // 2):
                nc.scalar.dma_start(
                    wg_sb[:, 2 * q : 2 * q + 2, :],
                    wg_d[2 * q * P : (2 * q + 2) * P, :].rearrange(
                        "(k p) f -> p k f", p=P
                    ),
                )
            fix_edges(xt0[:])
            dma_x(1)
            fix_edges(xt_tiles[1][:])
            dma_x(2)
            fix_edges(xt_tiles[2][:])
            nc.scalar.dma_start(b1_sb[:], b1_d.rearrange("(o n) -> o n", o=1))
            nc.scalar.dma_start(b2_sb[:], b2_d.rearrange("(o n) -> o n", o=1))
            nc.scalar.dma_start(id8_sb[:], id8_d[:])
            nc.vector.memset(ones_sb[:], 1.0)
            nc.scalar.dma_start(
                w1_sb[:], w1_d[:].rearrange("(i p) n -> p i n", p=P)
            )
            nc.scalar.dma_start(
                w2_sb[:], w2_d[:].rearrange("(m p) n -> p m n", p=P)
            )

            with (
                tc.tile_pool(
                    name=f"mpsum{_rep}", bufs=cfg.get("psum_bufs", 4), space="PSUM"
                ) as mpsum,
                tc.tile_pool(name=f"lpsum{_rep}", bufs=1, space="PSUM") as lpsum,
            ):
                hp = lpsum.tile([BL, H1], f32, tag="hp")

                def mlp_slice(c):
                    # first-layer matmuls for video v=c (pooled groups
                    # c*GR..(c+1)*GR are batches 0..BL-1 of video c)
                    for fc in range(KC):
                        i = c * KC + fc
                        nc.tensor.matmul(
                            hp[:],
                            pooled_m[:, fc, c * GR : (c + 1) * GR],
                            w1_sb[:, i, :],
                            start=(i == 0),
                            stop=False,
                        )

                # ---- chunks: j-outer Y-phase (fp8 DoubleRow, K=256 per
                # matmul) + reduce_max straight to bf16 + interleaved MLP ----
                xt_tiles[0] = xt0
                NQ = KC // 2
                for c in range(NCH):
                    if c + 2 < NCH and c > 0:
                        dma_x(c + 2)
                        fix_edges(xt_tiles[c + 2][:])
                    xt = xt_tiles.pop(c)
                    for j in range(JC):
                        yp = mpsum.tile([P, CHUNK], f32, tag="yp")
                        if cfg.get("double_row", True):
                            for q in range(NQ):
                                nc.tensor.matmul(
                                    yp[:],
                                    wg_sb[
                                        :, 2 * q : 2 * q + 2, j * P : (j + 1) * P
                                    ],
                                    xt[:, 2 * q : 2 * q + 2, :],
                                    start=(q == 0),
                                    stop=(q == NQ - 1),
                                    perf_mode=DR,
                                )
                        else:
                            for k in range(KC):
                                nc.tensor.matmul(
                                    yp[:],
                                    wg_sb[:, k, j * P : (j + 1) * P],
                                    xt[:, k, :],
                                    start=(k == 0),
                                    stop=(k == KC - 1),
                                )
                        nc.vector.reduce_max(
                            pooled_m[:, j, c * GR : (c + 1) * GR],
                            yp[:].rearrange("p (g t) -> p g t", t=T),
                            axis=mybir.AxisListType.X,
                        )
                    if c > 0 and cfg.get("mlp_interleave", True):
                        mlp_slice(c - 1)

                # ---- tail: last MLP slice, bias, relu, layer 2 ----
                if not cfg.get("mlp_interleave", True):
                    for c in range(NCH - 1):
                        mlp_slice(c)
                mlp_slice(NCH - 1)
                nc.tensor.matmul(hp[:], ones_sb[:], b1_sb[:], start=False, stop=True)
                h_sb = const.tile([BL, H1], f32)
                nc.scalar.activation(h_sb[:], hp[:], AF.Relu)

                ht_sb = const.tile([P, 2, BL], bf16)
                for m in range(2):
                    thp = lpsum.tile([P, BL], f32, tag="thp")
                    nc.tensor.transpose(
                        thp[:], h_sb[:, m * P : (m + 1) * P], id8_sb[:]
                    )
                    nc.vector.tensor_copy(ht_sb[:, m, :], thp[:])

                op = lpsum.tile([BL, OUT], f32, tag="op")
                for m in range(2):
                    nc.tensor.matmul(
                        op[:], ht_sb[:, m, :], w2_sb[:, m, :], start=(m == 0),
                        stop=False,
                    )
                nc.tensor.matmul(op[:], ones_sb[:], b2_sb[:], start=False, stop=True)
                o_sb = const.tile([BL, OUT], f32)
                nc.scalar.activation(o_sb[:], op[:], AF.Sigmoid)
                nc.sync.dma_start(
                    out_d[:] if reps == 1 else out_d[_rep], o_sb[:]
                )

    nc.compile()
    return nc


def _get_state(cfg=None):
    global _STATE
    if _STATE is None:
        _STATE = _build_nc(cfg or CFG)
    return _STATE


def make_in_maps(videos, W_gcn, b_gcn, W1, b1, W2, b2):
    videos = np.asarray(videos, dtype=np.float32)
    W1f = np.asarray(W1, dtype=np.float32)
    b_gcn = np.asarray(b_gcn, dtype=np.float32)
    # fold b_gcn into b1: relu((pooled + bg)@W1 + b1) = relu(pooled@W1 + b1')
    b1p = (np.asarray(b1, dtype=np.float32) + np.tile(b_gcn, V) @ W1f).astype(
        np.float32
    )
    id8 = np.eye(BL, dtype=np.float32)
    common = {
        "W_gcn": np.asarray(W_gcn, dtype=np.float32).astype(FP8),
        "W1": W1f.astype(BF16),
        "b1": b1p,
        "W2": np.asarray(W2, dtype=np.float32).astype(BF16),
        "b2": np.asarray(b2, dtype=np.float32),
        "id8": id8,
    }
    in_maps = []
    for i in range(NCORES):
        m = dict(common)
        # shard over batch; v-major node order; cast fp8; transpose to [F, NLOC]
        xc = videos[i * BL : (i + 1) * BL]  # [BL, V, T, F]
        xv = xc.transpose(1, 0, 2, 3).reshape(NLOC, F).astype(FP8)
        m["videosT"] = np.ascontiguousarray(xv.T)
        in_maps.append(m)
    return in_maps


_RUNNER = None


def _make_runner(nc):
    """Cached multi-core PJRT runner (mirrors bass2jax.run_bass_via_pjrt but
    jits once so repeated calls don't re-trace)."""
    import jax
    import numpy as _np
    from jax.experimental.shard_map import shard_map
    from jax.sharding import Mesh, PartitionSpec
    from concourse import bass2jax, mybir

    bass2jax.install_neuronx_cc_hook()
    assert nc.dbg_addr is None
    partition_name = (
        nc.partition_id_tensor.name if nc.partition_id_tensor is not None else None
    )

    in_names, out_names, out_avals, zero_outs = [], [], [], []
    for alloc in nc.m.functions[0].allocations:
        if not isinstance(alloc, mybir.MemoryLocationSet):
            continue
        name = alloc.memorylocations[0].name
        if alloc.kind == "ExternalInput":
            if name != partition_name:
                in_names.append(name)
        elif alloc.kind == "ExternalOutput":
            out_names.append(name)
            shape = tuple(alloc.tensor_shape)
            dtype = mybir.dt.np(alloc.dtype)
            out_avals.append(jax.core.ShapedArray(shape, dtype))
            zero_outs.append(_np.zeros(shape, dtype))
    n_params = len(in_names)
    n_outs = len(out_avals)
    all_names = in_names + out_names
    if partition_name is not None:
        all_names = all_names + [partition_name]

    def _body(*args):
        operands = list(args)
        if partition_name is not None:
            operands.append(bass2jax.partition_id_tensor())
        outs = bass2jax._bass_exec_p.bind(
            *operands,
            out_avals=tuple(out_avals),
            in_names=tuple(all_names),
            out_names=tuple(out_names),
            lowering_input_output_aliases=(),
            sim_require_finite=True,
            sim_require_nnan=True,
            nc=nc,
        )
        return tuple(outs)

    devices = jax.devices()[:NCORES]
    mesh = Mesh(np.asarray(devices), ("core",))
    in_specs = (PartitionSpec("core"),) * (n_params + n_outs)
    out_specs = (PartitionSpec("core"),) * n_outs
    sharded = jax.jit(
        shard_map(
            _body, mesh=mesh, in_specs=in_specs, out_specs=out_specs, check_rep=False
        ),
        keep_unused=True,
    )

    def run(in_maps, device_inputs=None):
        if device_inputs is None:
            device_inputs = prep(in_maps)
        out_arrs = sharded(*device_inputs)
        jax.block_until_ready(out_arrs)
        return [
            {
                name: _np.asarray(out_arrs[i]).reshape(NCORES, *out_avals[i].shape)[c]
                for i, name in enumerate(out_names)
            }
            for c in range(NCORES)
        ]

    def prep(in_maps):
        from jax.sharding import NamedSharding

        concat_in = [
            _np.concatenate([_np.asarray(in_maps[c][nm]) for c in range(NCORES)], 0)
            for nm in in_names
        ]
        concat_zeros = [
            _np.zeros((NCORES * z.shape[0], *z.shape[1:]), z.dtype) for z in zero_outs
        ]
        sh = NamedSharding(mesh, PartitionSpec("core"))
        arrs = [jax.device_put(a, sh) for a in concat_in + concat_zeros]
        jax.block_until_ready(arrs)
        return arrs

    return run, prep


def _get_runner():
    global _RUNNER
    if _RUNNER is None:
        _RUNNER = _make_runner(_get_state())
    return _RUNNER


def run_spmd(in_maps, device_inputs=None):
    run, _ = _get_runner()
    return run(in_maps, device_inputs)


def prep_inputs(in_maps):
    _, prep = _get_runner()
    return prep(in_maps)


def kernel(videos, W_gcn, b_gcn, W1, b1, W2, b2):
    in_maps = make_in_maps(videos, W_gcn, b_gcn, W1, b1, W2, b2)
    results = run_spmd(in_maps)
    out = np.stack([results[i]["out"] for i in range(NCORES)])  # [8, 8, 512]
    return out.reshape(B, OUT).reshape(B, V, T).astype(np.float32)
